# revision 28
# baseline (speedup 1.0000x reference)
"""ClassBalancedSupConLoss on 8 TRN2 NeuronCores (Bass/Tile).

Math (reference semantics, reorganized for hardware):
  - All embeddings are unit-norm, so s_ij = e_i . e_j <= 1 and s_ii ~= 1.
    Use a FIXED logsumexp shift m = 1:
        LSE_i = inv_t_i * 1 + log( sum_j exp(inv_t_i * (s_ij - 1)) )
    The self term is excluded by subtracting exp(inv_t*(s_ii-1)) where
    s_ii is computed ON DEVICE from the same rounded operands (bitwise
    identical to the self term inside the big sum, so the cancellation
    is exact even though matmul-input rounding makes s_ii != 1).
  - Batch and bank are sorted by class on the host, so the same-class
    column set of any anchor is one contiguous segment.  Bank same-class
    exclusion = (total exp sum) - (own-class segment exp sum); positives
    = (own-class raw-logit segment sum - s_ii) / pos_cnt.
  - Anchors (batch rows) are sharded 256/core across 8 cores; every core
    holds full embT/bankT replicas.  Per-anchor losses are DMA'd out;
    the final masked mean over 2048 anchors is a host-side reduction.

Engine structure per core (2 anchor tiles x [128 anchors]):
  - PE: S chunks [128, 512] into rotating [128, 2048] PSUM tiles
    (2 tiles x 4 banks).  bf16 inputs (fast FWL weight loads, 1 cyc/row).
  - ACT: one Exp pass per 2048-col PSUM chunk with accum_out row-sums;
    exp calls are SPLIT at class-segment boundaries, so per-class bank
    exp sums fall out of the per-call accumulators directly.
  - DVE: raw-logit segment reductions for positives + tiny epilogue.

SPMD: one program for all 8 cores.  Anything core-dependent (the anchor
slice, per-anchor temperature vectors, one-hot class rows) is passed as
per-core DATA; program constants (class segment boundaries) are global.
"""

import os
import numpy as np

import concourse.bass as bass  # noqa: F401
from concourse import bacc
import concourse.mybir as mybir
import concourse.tile as tile
from concourse.bass_utils import run_bass_kernel_spmd

B, D, M, C = 2048, 128, 16384, 3
NCORES = 8
APC = B // NCORES          # anchors per core = 256
NT = APC // 128            # anchor tiles per core = 2
CH = 512                   # matmul free chunk (one PSUM bank)
W = 2048                   # big PSUM chunk (4 banks) = one ACT Exp pass
NBK = M // W               # 8 bank pieces of [128, 2048]
BASE_TEMP = 0.07

F32 = mybir.dt.float32
AF = mybir.ActivationFunctionType
ALU = mybir.AluOpType
AX = mybir.AxisListType

# "bf16": matmul inputs bf16 (fast path; ~1e-3 logit rounding)
# "f32r": fp32 bits, PE rounds mantissa (slow LDWEIGHTS, ~4x PE time)
# "f32" : full fp32 matmul (4 cyc/row)
MM_MODE = os.environ.get("SUPCON_MM_MODE", "bf16")

LAST_EXEC_TIME_NS = None   # set by kernel() when SUPCON_TRACE=1


def _install_trace_shim():
    """Register the NTFF profile hook that this image's antenv lacks.

    Mirrors trn_agent_boot's _ntff_profile_via_ctypes: drives NRT
    profiling via the injected libaxon_pjrt.so.  Only used for local
    perf iteration (SUPCON_TRACE=1); the plain execution path never
    needs it.
    """
    import sys
    import types
    import ctypes
    import contextlib

    try:
        from antenv.axon_hooks import get_axon_ntff_profile_hook  # noqa: F401
        return True  # real module exists
    except ImportError:
        pass

    so_path = "/opt/axon/libaxon_pjrt.so"
    if not os.path.exists(so_path):
        return False
    lib = ctypes.CDLL(so_path)
    if not hasattr(lib, "axon_start_nrt_profile"):
        return False
    lib.axon_start_nrt_profile.argtypes = [
        ctypes.POINTER(ctypes.c_int64),
        ctypes.c_size_t,
    ]
    lib.axon_start_nrt_profile.restype = ctypes.c_int64
    lib.axon_stop_nrt_profile.argtypes = [ctypes.c_char_p]
    lib.axon_stop_nrt_profile.restype = ctypes.c_int64

    @contextlib.contextmanager
    def _hook(output_dir, device_ids):
        import jax

        jax.devices()
        if device_ids:
            ids = (ctypes.c_int64 * len(device_ids))(*device_ids)
            rc = lib.axon_start_nrt_profile(ids, len(device_ids))
        else:
            rc = lib.axon_start_nrt_profile(None, 0)
        if rc != 0:
            raise RuntimeError(f"axon_start_nrt_profile rc={rc}")
        try:
            yield
        finally:
            n = lib.axon_stop_nrt_profile(str(output_dir).encode())
            print(f"profile: {n} file(s) written to {output_dir}", file=sys.stderr)

    _state = {"hook": _hook}
    mod = types.ModuleType("antenv.axon_hooks")
    mod.get_axon_ntff_profile_hook = lambda: _state["hook"]
    mod.set_axon_ntff_profile_hook = lambda h: _state.update(hook=h)
    sys.modules["antenv.axon_hooks"] = mod
    import antenv

    antenv.axon_hooks = mod

    # skip the artifact upload (no bucket access needed for local iteration)
    import concourse.bass_utils as bu

    bu.upload_artifacts = lambda tmpdir: tmpdir
    return True


def _bank_subranges(mk_b1, mk_b2):
    """Split [0, M) at big-chunk multiples AND class boundaries.

    Returns (subs, i1, i2): subs = list of (start, end); i1/i2 = first
    subrange index at/after mk_b1/mk_b2 (class-segment column ranges in
    the per-subrange accumulator tile are then [0,i1), [i1,i2), [i2,n)).
    """
    cuts = sorted({c * W for c in range(NBK + 1)} | {mk_b1, mk_b2})
    subs = [(cuts[i], cuts[i + 1]) for i in range(len(cuts) - 1)]
    i1 = sum(1 for s, _ in subs if s < mk_b1)
    i2 = sum(1 for s, _ in subs if s < mk_b2)
    return subs, i1, i2


def _build(bb_b1, bb_b2, mk_b1, mk_b2, mm_mode):
    import ml_dtypes  # noqa: F401  (bf16 numpy dtype registration)

    if mm_mode == "bf16":
        in_dt = mybir.dt.bfloat16
    elif mm_mode == "f32":
        in_dt = F32
    else:
        in_dt = mybir.dt.float32r

    nc = bacc.Bacc()
    embT_d = nc.declare_dram_parameter("embT", [D, B], in_dt, isOutput=False)
    anchT_d = nc.declare_dram_parameter("anchT", [D, APC], in_dt, isOutput=False)
    bankT_d = nc.declare_dram_parameter("bankT", [D, M], in_dt, isOutput=False)
    subs, i1, i2 = _bank_subranges(mk_b1, mk_b2)
    NK = len(subs)
    # one packed small-vector input: [invt | ninvt | invpc | coefv | oneh |
    # incl | eye] along columns -- a single DMA instead of seven
    NV = NT * (4 + C + NK) + 128
    vecs_d = nc.declare_dram_parameter("vecs", [128, NV], F32, isOutput=False)
    oout_d = nc.declare_dram_parameter("oout", [128, 2 * NT], F32, isOutput=True)

    with tile.TileContext(nc) as tc:
        with (
            tc.tile_pool(name="big", bufs=1) as bigp,
            tc.tile_pool(name="sm", bufs=1) as smp,
            tc.tile_pool(name="scr", bufs=2) as scrp,
            tc.tile_pool(name="ps", bufs=2, space="PSUM") as psp,
        ):
            anch_t = bigp.tile([D, APC], in_dt, tag="anchT")
            vecs_t = smp.tile([128, NV], F32, tag="vecs")
            # garbage-operand warmup tiles (never written: no DMA dependency,
            # so the PE can start immediately and open the HAM clock gate)
            junkw_t = bigp.tile([128, 128], in_dt, tag="junkw")
            junkx_t = bigp.tile([128, CH], in_dt, tag="junkx")
            o = [0]
            def vslice(w):
                a = o[0]; o[0] += w
                return vecs_t[:, a:a + w]
            invt_t = vslice(NT)
            ninvt_t = vslice(NT)
            invpc_t = vslice(NT)
            coefv_t = vslice(NT)
            oneh_t = vslice(NT * C)
            incl_t = vslice(NT * NK)
            eye_t = vslice(128)
            # spread transfers over both HWDGE queues (sync, scalar) and the
            # gpsimd SWDGE queues so nothing serializes behind emb
            emb_t = bigp.tile([D, B], in_dt, tag="embT")
            nc.sync.dma_start(out=emb_t[:, 0:B // 2], in_=embT_d[:, 0:B // 2])
            nc.scalar.dma_start(out=emb_t[:, B // 2:B], in_=embT_d[:, B // 2:B])
            nc.scalar.dma_start(out=anch_t[:], in_=anchT_d[:])
            nc.scalar.dma_start(out=vecs_t[:], in_=vecs_d[:])
            bank_ts = []
            for j in range(NBK):
                bt = bigp.tile([D, W], in_dt, tag=f"bank{j}", name=f"bank{j}")
                if j < 2:
                    nc.sync.dma_start(out=bt[:], in_=bankT_d[:, j * W:(j + 1) * W])
                else:
                    nc.gpsimd.dma_start(out=bt[:], in_=bankT_d[:, j * W:(j + 1) * W])
                bank_ts.append(bt)

            oout_t = smp.tile([128, 2 * NT], F32, tag="oout")
            sdiag = [smp.tile([128, 1], F32, tag=f"sdiag{t}", name=f"sdiag{t}") for t in range(NT)]
            selfe = [smp.tile([128, 1], F32, tag=f"selfe{t}", name=f"selfe{t}") for t in range(NT)]
            eyemul = smp.tile([128, 128], F32, tag="eyemul")
            warm = smp.tile([128, 1], F32, tag="warm")
            bbsum = [smp.tile([128, 1], F32, tag=f"bbsum{t}", name=f"bbsum{t}") for t in range(NT)]
            raw3 = [smp.tile([128, C], F32, tag=f"raw3{t}", name=f"raw3{t}") for t in range(NT)]
            esum = [smp.tile([128, NK], F32, tag=f"esum{t}", name=f"esum{t}") for t in range(NT)]

            # pull the Exp table load off the critical path
            nc.scalar.activation(warm[:], eye_t[:, 0:1], AF.Exp)

            def anch(t):
                return anch_t[:, t * 128:(t + 1) * 128]

            # ~4.3us of contiguous PE activity before the DMAs land: HAM
            # un-throttles (1.2 -> 2.4 GHz) before the real stream begins
            nc.vector.memset(junkw_t[:], 0.0)
            nc.vector.memset(junkx_t[:], 0.0)
            warm_ps = psp.tile([128, W], F32, tag="chunk", name="warm_ps")
            for w in range(12):
                nc.tensor.matmul(
                    warm_ps[:, (w % 4) * CH:((w % 4) + 1) * CH],
                    junkw_t[:], junkx_t[:], start=True, stop=True,
                )

            # ---- prelude: self-similarity blocks (diag -> s_ii) ----
            pre_ps = psp.tile([128, W], F32, tag="chunk", name="pre_ps")
            for t in range(NT):
                nc.tensor.matmul(
                    pre_ps[:, t * 128:(t + 1) * 128], anch(t), anch(t),
                    start=True, stop=True,
                )
            for t in range(NT):
                nc.vector.tensor_mul(eyemul[:], pre_ps[:, t * 128:(t + 1) * 128], eye_t[:])
                nc.vector.reduce_sum(sdiag[t][:], eyemul[:], axis=AX.X)
                nc.scalar.activation(
                    selfe[t][:], sdiag[t][:], AF.Exp,
                    bias=ninvt_t[:, t:t + 1], scale=invt_t[:, t:t + 1],
                )

            by_chunk = {}
            for k, (s, e) in enumerate(subs):
                by_chunk.setdefault(s // W, []).append((s, e, k))

            scrNK = [smp.tile([128, NK], F32, tag=f"scrNK{t}", name=f"scrNK{t}") for t in range(NT)]
            scrC = [smp.tile([128, C], F32, tag=f"scrC{t}", name=f"scrC{t}") for t in range(NT)]

            def epilogue(t):
                """Fused DVE chain; log() finishes on the host.
                oden = denominator sum; olin = coefv*invt*(1 - pos)."""
                p1 = smp.tile([128, 1], F32, tag=f"p1{t}", name=f"p1{t}")
                own_r = smp.tile([128, 1], F32, tag=f"ownr{t}", name=f"ownr{t}")
                pos = smp.tile([128, 1], F32, tag=f"pos{t}", name=f"pos{t}")
                w1 = smp.tile([128, 1], F32, tag=f"w1{t}", name=f"w1{t}")
                # den = (bbsum - selfe) + sum_k esum_k * incl_k
                nc.vector.tensor_sub(p1[:], bbsum[t][:], selfe[t][:])
                nc.vector.tensor_mul(scrNK[t][:], esum[t][:], incl_t[:, t * NK:(t + 1) * NK])
                nc.vector.reduce_sum(oout_t[:, t:t + 1], scrNK[t][:], axis=AX.X)
                nc.vector.tensor_add(oout_t[:, t:t + 1], oout_t[:, t:t + 1], p1[:])
                # own_r = sum_c raw3_c * oneh_c
                nc.vector.tensor_mul(scrC[t][:], raw3[t][:], oneh_t[:, t * C:(t + 1) * C])
                nc.vector.reduce_sum(own_r[:], scrC[t][:], axis=AX.X)
                # olin = coefv * invt * (1 - (own_r - s_ii)*invpc)
                nc.vector.scalar_tensor_tensor(
                    out=pos[:], in0=own_r[:], scalar=sdiag[t][:], in1=invpc_t[:, t:t + 1],
                    op0=ALU.subtract, op1=ALU.mult,
                )
                nc.vector.scalar_tensor_tensor(
                    out=w1[:], in0=pos[:], scalar=-1.0, in1=invt_t[:, t:t + 1],
                    op0=ALU.mult, op1=ALU.mult,
                )
                nc.vector.scalar_tensor_tensor(
                    out=oout_t[:, NT + t:NT + t + 1], in0=w1[:], scalar=invt_t[:, t:t + 1],
                    in1=coefv_t[:, t:t + 1], op0=ALU.add, op1=ALU.mult,
                )

            def emit_bb(t):
                ps = psp.tile([128, W], F32, tag="chunk", name="bb_ps")
                for q in range(W // CH):
                    nc.tensor.matmul(
                        ps[:, q * CH:(q + 1) * CH], anch(t),
                        emb_t[:, q * CH:(q + 1) * CH],
                        start=True, stop=True,
                    )
                scr = scrp.tile([128, W], F32, tag="scr", name="scr_bb")
                nc.scalar.activation(
                    scr[:], ps[:], AF.Exp,
                    bias=ninvt_t[:, t:t + 1], scale=invt_t[:, t:t + 1],
                    accum_out=bbsum[t][:],
                )
                nc.vector.reduce_sum(raw3[t][:, 0:1], ps[:, 0:bb_b1], axis=AX.X)
                nc.vector.reduce_sum(raw3[t][:, 1:2], ps[:, bb_b1:bb_b2], axis=AX.X)
                nc.vector.reduce_sum(raw3[t][:, 2:3], ps[:, bb_b2:B], axis=AX.X)

            def emit_bank(t, j):
                ps = psp.tile([128, W], F32, tag="chunk", name="bk_ps")
                for q in range(W // CH):
                    nc.tensor.matmul(
                        ps[:, q * CH:(q + 1) * CH], anch(t),
                        bank_ts[j][:, q * CH:(q + 1) * CH],
                        start=True, stop=True,
                    )
                scr = scrp.tile([128, W], F32, tag="scr", name="scr_bk")
                for (s, e, k) in by_chunk[j]:
                    a, b = s - j * W, e - j * W
                    nc.scalar.activation(
                        scr[:, a:b], ps[:, a:b], AF.Exp,
                        bias=ninvt_t[:, t:t + 1], scale=invt_t[:, t:t + 1],
                        accum_out=esum[t][:, k:k + 1],
                    )

            # all of t0 (its DVE-only epilogue overlaps t1's stream); t1's
            # first chunk emitted before t0's last so PE never drains
            emit_bb(0)
            for j in range(NBK - 1):
                emit_bank(0, j)
            emit_bb(1)
            emit_bank(0, NBK - 1)
            epilogue(0)
            for j in range(NBK):
                emit_bank(1, j)
            epilogue(1)

            nc.sync.dma_start(out=oout_d[:], in_=oout_t[:])

    nc.compile()
    return nc


def _per_core_cols(vec, core):
    """[B] host vector -> [128, NT] tile for one core (col t, partition p)."""
    sl = vec[core * APC:(core + 1) * APC]
    return np.ascontiguousarray(sl.reshape(NT, 128).T).astype(np.float32)


def kernel(embeddings, labels, bank_embs, bank_labels, class_temps):
    global LAST_EXEC_TIME_NS
    import ml_dtypes

    emb = np.asarray(embeddings, dtype=np.float32)
    bank = np.asarray(bank_embs, dtype=np.float32)
    lab = np.asarray(labels).astype(np.int64).ravel()
    blab = np.asarray(bank_labels).astype(np.int64).ravel()
    ct = np.asarray(class_temps, dtype=np.float32).ravel()

    bord = np.argsort(lab, kind="stable")
    slab = lab[bord]
    mord = np.argsort(blab, kind="stable")
    cnt = np.bincount(lab, minlength=C)
    mcnt = np.bincount(blab, minlength=C)
    bb_b1, bb_b2 = int(cnt[0]), int(cnt[0] + cnt[1])
    mk_b1, mk_b2 = int(mcnt[0]), int(mcnt[0] + mcnt[1])

    embT = np.ascontiguousarray(emb[bord].T)      # [D, B]
    bankT = np.ascontiguousarray(bank[mord].T)    # [D, M]
    if MM_MODE == "bf16":
        embT = embT.astype(ml_dtypes.bfloat16)
        bankT = bankT.astype(ml_dtypes.bfloat16)

    temps = ct[slab]
    inv_t = (1.0 / temps).astype(np.float32)
    pos_cnt = cnt[slab] - 1
    invpc = (1.0 / np.maximum(pos_cnt, 1)).astype(np.float32)
    validf = (pos_cnt > 0).astype(np.float32)
    coefv = (BASE_TEMP / temps).astype(np.float32) * validf
    oneh = np.eye(C, dtype=np.float32)[slab]      # [B, 3]
    n_valid = int((pos_cnt > 0).sum())

    nc = _build(bb_b1, bb_b2, mk_b1, mk_b2, MM_MODE)

    subs, _, _ = _bank_subranges(mk_b1, mk_b2)
    NK = len(subs)
    sub_cls = np.array([0 if s < mk_b1 else (1 if s < mk_b2 else 2) for s, _ in subs])
    # incl[anchor, k] = 1 where subrange class != anchor class
    incl_full = (sub_cls[None, :] != slab[:, None]).astype(np.float32)  # [B, NK]
    eye128 = np.eye(128, dtype=np.float32)

    in_maps = []
    for core in range(NCORES):
        asl = slice(core * APC, (core + 1) * APC)
        oh = oneh[asl].reshape(NT, 128, C).transpose(1, 0, 2).reshape(128, NT * C)
        ic = incl_full[asl].reshape(NT, 128, NK).transpose(1, 0, 2).reshape(128, NT * NK)
        vecs = np.concatenate([
            _per_core_cols(inv_t, core),
            _per_core_cols(-inv_t, core),
            _per_core_cols(invpc, core),
            _per_core_cols(coefv, core),
            oh.astype(np.float32),
            ic.astype(np.float32),
            eye128,
        ], axis=1)
        in_maps.append({
            "embT": embT,
            "anchT": np.ascontiguousarray(embT[:, asl]),
            "bankT": bankT,
            "vecs": np.ascontiguousarray(vecs),
        })

    trace = os.environ.get("SUPCON_TRACE", "0") == "1"
    if trace:
        trace = _install_trace_shim()
    res = run_bass_kernel_spmd(nc, in_maps, core_ids=list(range(NCORES)), trace=trace)
    LAST_EXEC_TIME_NS = res.exec_time_ns

    # loss_i = coef_i * log(den_i) + lin_i ; device produced den/lin,
    # host finishes the 2048 scalar logs + masked mean
    loss_sum = np.float64(0.0)
    for core in range(NCORES):
        oo = np.asarray(res.results[core]["oout"], dtype=np.float64)    # [128, 2*NT]
        den, lin = oo[:, :NT], oo[:, NT:]
        cf = _per_core_cols(coefv, core).astype(np.float64)
        loss_sum += (cf * np.log(den) + lin).sum()
    return np.float32(loss_sum / max(n_valid, 1))


# revision 29
# speedup vs baseline: 1.0614x; 1.0614x over previous
"""ClassBalancedSupConLoss on 8 TRN2 NeuronCores (Bass/Tile).

Math (reference semantics, reorganized for hardware):
  - All embeddings are unit-norm, so s_ij = e_i . e_j <= 1 and s_ii ~= 1.
    Use a FIXED logsumexp shift m = 1:
        LSE_i = inv_t_i * 1 + log( sum_j exp(inv_t_i * (s_ij - 1)) )
    The self term is excluded by subtracting exp(inv_t*(s_ii-1)) where
    s_ii is computed ON DEVICE from the same rounded operands (bitwise
    identical to the self term inside the big sum, so the cancellation
    is exact even though matmul-input rounding makes s_ii != 1).
  - Batch and bank are sorted by class on the host, so the same-class
    column set of any anchor is one contiguous segment.  Bank same-class
    exclusion = (total exp sum) - (own-class segment exp sum); positives
    = (own-class raw-logit segment sum - s_ii) / pos_cnt.
  - Anchors (batch rows) are sharded 256/core across 8 cores; every core
    holds full embT/bankT replicas.  Per-anchor losses are DMA'd out;
    the final masked mean over 2048 anchors is a host-side reduction.

Engine structure per core (2 anchor tiles x [128 anchors]):
  - PE: S chunks [128, 512] into rotating [128, 2048] PSUM tiles
    (2 tiles x 4 banks).  bf16 inputs (fast FWL weight loads, 1 cyc/row).
  - ACT: one Exp pass per 2048-col PSUM chunk with accum_out row-sums;
    exp calls are SPLIT at class-segment boundaries, so per-class bank
    exp sums fall out of the per-call accumulators directly.
  - DVE: raw-logit segment reductions for positives + tiny epilogue.

SPMD: one program for all 8 cores.  Anything core-dependent (the anchor
slice, per-anchor temperature vectors, one-hot class rows) is passed as
per-core DATA; program constants (class segment boundaries) are global.
"""

import os
import numpy as np

import concourse.bass as bass  # noqa: F401
from concourse import bacc
import concourse.mybir as mybir
import concourse.tile as tile
from concourse.bass_utils import run_bass_kernel_spmd

B, D, M, C = 2048, 128, 16384, 3
NCORES = 8
APC = B // NCORES          # anchors per core = 256
NT = APC // 128            # anchor tiles per core = 2
CH = 512                   # matmul free chunk (one PSUM bank)
W = 2048                   # big PSUM chunk (4 banks) = one ACT Exp pass
NBK = M // W               # 8 bank pieces of [128, 2048]
BASE_TEMP = 0.07

F32 = mybir.dt.float32
AF = mybir.ActivationFunctionType
ALU = mybir.AluOpType
AX = mybir.AxisListType

# "bf16": matmul inputs bf16 (fast path; ~1e-3 logit rounding)
# "f32r": fp32 bits, PE rounds mantissa (slow LDWEIGHTS, ~4x PE time)
# "f32" : full fp32 matmul (4 cyc/row)
MM_MODE = os.environ.get("SUPCON_MM_MODE", "bf16")

LAST_EXEC_TIME_NS = None   # set by kernel() when SUPCON_TRACE=1


def _install_trace_shim():
    """Register the NTFF profile hook that this image's antenv lacks.

    Mirrors trn_agent_boot's _ntff_profile_via_ctypes: drives NRT
    profiling via the injected libaxon_pjrt.so.  Only used for local
    perf iteration (SUPCON_TRACE=1); the plain execution path never
    needs it.
    """
    import sys
    import types
    import ctypes
    import contextlib

    try:
        from antenv.axon_hooks import get_axon_ntff_profile_hook  # noqa: F401
        return True  # real module exists
    except ImportError:
        pass

    so_path = "/opt/axon/libaxon_pjrt.so"
    if not os.path.exists(so_path):
        return False
    lib = ctypes.CDLL(so_path)
    if not hasattr(lib, "axon_start_nrt_profile"):
        return False
    lib.axon_start_nrt_profile.argtypes = [
        ctypes.POINTER(ctypes.c_int64),
        ctypes.c_size_t,
    ]
    lib.axon_start_nrt_profile.restype = ctypes.c_int64
    lib.axon_stop_nrt_profile.argtypes = [ctypes.c_char_p]
    lib.axon_stop_nrt_profile.restype = ctypes.c_int64

    @contextlib.contextmanager
    def _hook(output_dir, device_ids):
        import jax

        jax.devices()
        if device_ids:
            ids = (ctypes.c_int64 * len(device_ids))(*device_ids)
            rc = lib.axon_start_nrt_profile(ids, len(device_ids))
        else:
            rc = lib.axon_start_nrt_profile(None, 0)
        if rc != 0:
            raise RuntimeError(f"axon_start_nrt_profile rc={rc}")
        try:
            yield
        finally:
            n = lib.axon_stop_nrt_profile(str(output_dir).encode())
            print(f"profile: {n} file(s) written to {output_dir}", file=sys.stderr)

    _state = {"hook": _hook}
    mod = types.ModuleType("antenv.axon_hooks")
    mod.get_axon_ntff_profile_hook = lambda: _state["hook"]
    mod.set_axon_ntff_profile_hook = lambda h: _state.update(hook=h)
    sys.modules["antenv.axon_hooks"] = mod
    import antenv

    antenv.axon_hooks = mod

    # skip the artifact upload (no bucket access needed for local iteration)
    import concourse.bass_utils as bu

    bu.upload_artifacts = lambda tmpdir: tmpdir
    return True


def _bank_subranges(mk_b1, mk_b2):
    """Split [0, M) at big-chunk multiples AND class boundaries.

    Returns (subs, i1, i2): subs = list of (start, end); i1/i2 = first
    subrange index at/after mk_b1/mk_b2 (class-segment column ranges in
    the per-subrange accumulator tile are then [0,i1), [i1,i2), [i2,n)).
    """
    cuts = sorted({c * W for c in range(NBK + 1)} | {mk_b1, mk_b2})
    subs = [(cuts[i], cuts[i + 1]) for i in range(len(cuts) - 1)]
    i1 = sum(1 for s, _ in subs if s < mk_b1)
    i2 = sum(1 for s, _ in subs if s < mk_b2)
    return subs, i1, i2


def _build(bb_b1, bb_b2, mk_b1, mk_b2, mm_mode):
    import ml_dtypes  # noqa: F401  (bf16 numpy dtype registration)

    if mm_mode == "bf16":
        in_dt = mybir.dt.bfloat16
    elif mm_mode == "f32":
        in_dt = F32
    else:
        in_dt = mybir.dt.float32r

    nc = bacc.Bacc()
    embT_d = nc.declare_dram_parameter("embT", [D, B], in_dt, isOutput=False)
    anchT_d = nc.declare_dram_parameter("anchT", [D, APC], in_dt, isOutput=False)
    bankT_d = nc.declare_dram_parameter("bankT", [D, M], in_dt, isOutput=False)
    subs, i1, i2 = _bank_subranges(mk_b1, mk_b2)
    NK = len(subs)
    # one packed small-vector input: [invt | ninvt | invpc | coefv | oneh |
    # incl | eye] along columns -- a single DMA instead of seven
    NV = NT * (4 + C + NK) + 128
    vecs_d = nc.declare_dram_parameter("vecs", [128, NV], F32, isOutput=False)
    oout_d = nc.declare_dram_parameter("oout", [128, 2 * NT], F32, isOutput=True)

    with tile.TileContext(nc) as tc:
        with (
            tc.tile_pool(name="big", bufs=1) as bigp,
            tc.tile_pool(name="sm", bufs=1) as smp,
            tc.tile_pool(name="scr", bufs=2) as scrp,
            tc.tile_pool(name="ps", bufs=2, space="PSUM") as psp,
        ):
            anch_t = bigp.tile([D, APC], in_dt, tag="anchT")
            vecs_t = smp.tile([128, NV], F32, tag="vecs")
            # garbage-operand warmup tiles (never written: no DMA dependency,
            # so the PE can start immediately and open the HAM clock gate)
            junkw_t = bigp.tile([128, 128], in_dt, tag="junkw")
            junkx_t = bigp.tile([128, CH], in_dt, tag="junkx")
            o = [0]
            def vslice(w):
                a = o[0]; o[0] += w
                return vecs_t[:, a:a + w]
            invt_t = vslice(NT)
            ninvt_t = vslice(NT)
            invpc_t = vslice(NT)
            coefv_t = vslice(NT)
            oneh_t = vslice(NT * C)
            incl_t = vslice(NT * NK)
            eye_t = vslice(128)
            # both HWDGE queues (sync + scalar), pieces ordered by the time
            # the chunk stream consumes them; early pieces split in halves so
            # they complete sooner
            emb_t = bigp.tile([D, B], in_dt, tag="embT")
            bank_ts = [bigp.tile([D, W], in_dt, tag=f"bank{j}", name=f"bank{j}")
                       for j in range(NBK)]
            H = B // 2
            nc.scalar.dma_start(out=anch_t[:], in_=anchT_d[:])
            nc.sync.dma_start(out=emb_t[:, 0:H], in_=embT_d[:, 0:H])
            nc.scalar.dma_start(out=emb_t[:, H:B], in_=embT_d[:, H:B])
            nc.sync.dma_start(out=bank_ts[0][:, 0:H], in_=bankT_d[:, 0:H])
            nc.scalar.dma_start(out=bank_ts[0][:, H:W], in_=bankT_d[:, H:W])
            nc.sync.dma_start(out=bank_ts[1][:, 0:H], in_=bankT_d[:, W:W + H])
            nc.scalar.dma_start(out=bank_ts[1][:, H:W], in_=bankT_d[:, W + H:2 * W])
            nc.sync.dma_start(out=vecs_t[:], in_=vecs_d[:])
            for j in range(2, NBK):
                eng = nc.sync if j % 2 == 0 else nc.scalar
                eng.dma_start(out=bank_ts[j][:], in_=bankT_d[:, j * W:(j + 1) * W])

            oout_t = smp.tile([128, 2 * NT], F32, tag="oout")
            sdiag = [smp.tile([128, 1], F32, tag=f"sdiag{t}", name=f"sdiag{t}") for t in range(NT)]
            selfe = [smp.tile([128, 1], F32, tag=f"selfe{t}", name=f"selfe{t}") for t in range(NT)]
            eyemul = smp.tile([128, 128], F32, tag="eyemul")
            warm = smp.tile([128, 1], F32, tag="warm")
            bbsum = [smp.tile([128, 1], F32, tag=f"bbsum{t}", name=f"bbsum{t}") for t in range(NT)]
            raw3 = [smp.tile([128, C], F32, tag=f"raw3{t}", name=f"raw3{t}") for t in range(NT)]
            esum = [smp.tile([128, NK], F32, tag=f"esum{t}", name=f"esum{t}") for t in range(NT)]

            # pull the Exp table load off the critical path
            nc.scalar.activation(warm[:], eye_t[:, 0:1], AF.Exp)

            def anch(t):
                return anch_t[:, t * 128:(t + 1) * 128]

            # ~4.3us of contiguous PE activity before the DMAs land: HAM
            # un-throttles (1.2 -> 2.4 GHz) before the real stream begins
            nc.vector.memset(junkw_t[:], 0.0)
            nc.vector.memset(junkx_t[:], 0.0)
            warm_ps = psp.tile([128, W], F32, tag="chunk", name="warm_ps")
            for w in range(12):
                nc.tensor.matmul(
                    warm_ps[:, (w % 4) * CH:((w % 4) + 1) * CH],
                    junkw_t[:], junkx_t[:], start=True, stop=True,
                )

            # ---- prelude: self-similarity blocks (diag -> s_ii) ----
            pre_ps = psp.tile([128, W], F32, tag="chunk", name="pre_ps")
            for t in range(NT):
                nc.tensor.matmul(
                    pre_ps[:, t * 128:(t + 1) * 128], anch(t), anch(t),
                    start=True, stop=True,
                )
            for t in range(NT):
                nc.vector.tensor_mul(eyemul[:], pre_ps[:, t * 128:(t + 1) * 128], eye_t[:])
                nc.vector.reduce_sum(sdiag[t][:], eyemul[:], axis=AX.X)
                nc.scalar.activation(
                    selfe[t][:], sdiag[t][:], AF.Exp,
                    bias=ninvt_t[:, t:t + 1], scale=invt_t[:, t:t + 1],
                )

            by_chunk = {}
            for k, (s, e) in enumerate(subs):
                by_chunk.setdefault(s // W, []).append((s, e, k))

            scrNK = [smp.tile([128, NK], F32, tag=f"scrNK{t}", name=f"scrNK{t}") for t in range(NT)]
            scrC = [smp.tile([128, C], F32, tag=f"scrC{t}", name=f"scrC{t}") for t in range(NT)]

            def epilogue(t):
                """Fused DVE chain; log() finishes on the host.
                oden = denominator sum; olin = coefv*invt*(1 - pos)."""
                p1 = smp.tile([128, 1], F32, tag=f"p1{t}", name=f"p1{t}")
                own_r = smp.tile([128, 1], F32, tag=f"ownr{t}", name=f"ownr{t}")
                pos = smp.tile([128, 1], F32, tag=f"pos{t}", name=f"pos{t}")
                w1 = smp.tile([128, 1], F32, tag=f"w1{t}", name=f"w1{t}")
                # den = (bbsum - selfe) + sum_k esum_k * incl_k
                nc.vector.tensor_sub(p1[:], bbsum[t][:], selfe[t][:])
                nc.vector.tensor_mul(scrNK[t][:], esum[t][:], incl_t[:, t * NK:(t + 1) * NK])
                nc.vector.reduce_sum(oout_t[:, t:t + 1], scrNK[t][:], axis=AX.X)
                nc.vector.tensor_add(oout_t[:, t:t + 1], oout_t[:, t:t + 1], p1[:])
                # own_r = sum_c raw3_c * oneh_c
                nc.vector.tensor_mul(scrC[t][:], raw3[t][:], oneh_t[:, t * C:(t + 1) * C])
                nc.vector.reduce_sum(own_r[:], scrC[t][:], axis=AX.X)
                # olin = coefv * invt * (1 - (own_r - s_ii)*invpc)
                nc.vector.scalar_tensor_tensor(
                    out=pos[:], in0=own_r[:], scalar=sdiag[t][:], in1=invpc_t[:, t:t + 1],
                    op0=ALU.subtract, op1=ALU.mult,
                )
                nc.vector.scalar_tensor_tensor(
                    out=w1[:], in0=pos[:], scalar=-1.0, in1=invt_t[:, t:t + 1],
                    op0=ALU.mult, op1=ALU.mult,
                )
                nc.vector.scalar_tensor_tensor(
                    out=oout_t[:, NT + t:NT + t + 1], in0=w1[:], scalar=invt_t[:, t:t + 1],
                    in1=coefv_t[:, t:t + 1], op0=ALU.add, op1=ALU.mult,
                )

            def emit_bb(t):
                ps = psp.tile([128, W], F32, tag="chunk", name="bb_ps")
                for q in range(W // CH):
                    nc.tensor.matmul(
                        ps[:, q * CH:(q + 1) * CH], anch(t),
                        emb_t[:, q * CH:(q + 1) * CH],
                        start=True, stop=True,
                    )
                scr = scrp.tile([128, W], F32, tag="scr", name="scr_bb")
                nc.scalar.activation(
                    scr[:], ps[:], AF.Exp,
                    bias=ninvt_t[:, t:t + 1], scale=invt_t[:, t:t + 1],
                    accum_out=bbsum[t][:],
                )
                nc.vector.reduce_sum(raw3[t][:, 0:1], ps[:, 0:bb_b1], axis=AX.X)
                nc.vector.reduce_sum(raw3[t][:, 1:2], ps[:, bb_b1:bb_b2], axis=AX.X)
                nc.vector.reduce_sum(raw3[t][:, 2:3], ps[:, bb_b2:B], axis=AX.X)

            def emit_bank(t, j):
                ps = psp.tile([128, W], F32, tag="chunk", name="bk_ps")
                for q in range(W // CH):
                    nc.tensor.matmul(
                        ps[:, q * CH:(q + 1) * CH], anch(t),
                        bank_ts[j][:, q * CH:(q + 1) * CH],
                        start=True, stop=True,
                    )
                scr = scrp.tile([128, W], F32, tag="scr", name="scr_bk")
                for (s, e, k) in by_chunk[j]:
                    a, b = s - j * W, e - j * W
                    nc.scalar.activation(
                        scr[:, a:b], ps[:, a:b], AF.Exp,
                        bias=ninvt_t[:, t:t + 1], scale=invt_t[:, t:t + 1],
                        accum_out=esum[t][:, k:k + 1],
                    )

            # all of t0 (its DVE-only epilogue overlaps t1's stream); t1's
            # first chunk emitted before t0's last so PE never drains
            emit_bb(0)
            for j in range(NBK - 1):
                emit_bank(0, j)
            emit_bb(1)
            emit_bank(0, NBK - 1)
            epilogue(0)
            for j in range(NBK):
                emit_bank(1, j)
            epilogue(1)

            nc.sync.dma_start(out=oout_d[:], in_=oout_t[:])

    nc.compile()
    return nc


def _per_core_cols(vec, core):
    """[B] host vector -> [128, NT] tile for one core (col t, partition p)."""
    sl = vec[core * APC:(core + 1) * APC]
    return np.ascontiguousarray(sl.reshape(NT, 128).T).astype(np.float32)


def kernel(embeddings, labels, bank_embs, bank_labels, class_temps):
    global LAST_EXEC_TIME_NS
    import ml_dtypes

    emb = np.asarray(embeddings, dtype=np.float32)
    bank = np.asarray(bank_embs, dtype=np.float32)
    lab = np.asarray(labels).astype(np.int64).ravel()
    blab = np.asarray(bank_labels).astype(np.int64).ravel()
    ct = np.asarray(class_temps, dtype=np.float32).ravel()

    bord = np.argsort(lab, kind="stable")
    slab = lab[bord]
    mord = np.argsort(blab, kind="stable")
    cnt = np.bincount(lab, minlength=C)
    mcnt = np.bincount(blab, minlength=C)
    bb_b1, bb_b2 = int(cnt[0]), int(cnt[0] + cnt[1])
    mk_b1, mk_b2 = int(mcnt[0]), int(mcnt[0] + mcnt[1])

    embT = np.ascontiguousarray(emb[bord].T)      # [D, B]
    bankT = np.ascontiguousarray(bank[mord].T)    # [D, M]
    if MM_MODE == "bf16":
        embT = embT.astype(ml_dtypes.bfloat16)
        bankT = bankT.astype(ml_dtypes.bfloat16)

    temps = ct[slab]
    inv_t = (1.0 / temps).astype(np.float32)
    pos_cnt = cnt[slab] - 1
    invpc = (1.0 / np.maximum(pos_cnt, 1)).astype(np.float32)
    validf = (pos_cnt > 0).astype(np.float32)
    coefv = (BASE_TEMP / temps).astype(np.float32) * validf
    oneh = np.eye(C, dtype=np.float32)[slab]      # [B, 3]
    n_valid = int((pos_cnt > 0).sum())

    nc = _build(bb_b1, bb_b2, mk_b1, mk_b2, MM_MODE)

    subs, _, _ = _bank_subranges(mk_b1, mk_b2)
    NK = len(subs)
    sub_cls = np.array([0 if s < mk_b1 else (1 if s < mk_b2 else 2) for s, _ in subs])
    # incl[anchor, k] = 1 where subrange class != anchor class
    incl_full = (sub_cls[None, :] != slab[:, None]).astype(np.float32)  # [B, NK]
    eye128 = np.eye(128, dtype=np.float32)

    in_maps = []
    for core in range(NCORES):
        asl = slice(core * APC, (core + 1) * APC)
        oh = oneh[asl].reshape(NT, 128, C).transpose(1, 0, 2).reshape(128, NT * C)
        ic = incl_full[asl].reshape(NT, 128, NK).transpose(1, 0, 2).reshape(128, NT * NK)
        vecs = np.concatenate([
            _per_core_cols(inv_t, core),
            _per_core_cols(-inv_t, core),
            _per_core_cols(invpc, core),
            _per_core_cols(coefv, core),
            oh.astype(np.float32),
            ic.astype(np.float32),
            eye128,
        ], axis=1)
        in_maps.append({
            "embT": embT,
            "anchT": np.ascontiguousarray(embT[:, asl]),
            "bankT": bankT,
            "vecs": np.ascontiguousarray(vecs),
        })

    trace = os.environ.get("SUPCON_TRACE", "0") == "1"
    if trace:
        trace = _install_trace_shim()
    res = run_bass_kernel_spmd(nc, in_maps, core_ids=list(range(NCORES)), trace=trace)
    LAST_EXEC_TIME_NS = res.exec_time_ns

    # loss_i = coef_i * log(den_i) + lin_i ; device produced den/lin,
    # host finishes the 2048 scalar logs + masked mean
    loss_sum = np.float64(0.0)
    for core in range(NCORES):
        oo = np.asarray(res.results[core]["oout"], dtype=np.float64)    # [128, 2*NT]
        den, lin = oo[:, :NT], oo[:, NT:]
        cf = _per_core_cols(coefv, core).astype(np.float64)
        loss_sum += (cf * np.log(den) + lin).sum()
    return np.float32(loss_sum / max(n_valid, 1))
